# revision 62
# baseline (speedup 1.0000x reference)
"""GRU observation-cell kernel for Trainium2 (8 NeuronCores).

Reference computation:
    x = X_obs.reshape(M, 128); hs = h[i_obs]
    h_new = GRUCell(x, hs)  (torch gate order r,z,n)
    out = h.at[i_obs].set(h_new)

Device strategy (data parallel over observed rows, per sharding hint):
  - M=20000 observed rows sharded 2500/core across 8 cores, 5 column-tiles
    of 500 rows per core (gates-on-partitions layout).
  - r/z gates: fp8e4 DoubleRow matmuls (0.5 cyc/row). The moving operand
    holds pairs (x|const, h_lo|h_hi); the const group carries a 1.0 row so
    all biases ride the matmul as an extra contract row. PSUM then holds
    16*(gates+bias) and one sigmoid per gate pair drains two PSUM banks
    with scale=1/16 (weights are pre-scaled by 16 for fp8 subnormal
    headroom; a per-partition activation bias could not express per-bank
    biases, the const-row trick can).
  - n gate: i_n via one fp8 DoubleRow matmul; h_n in fp16 (tanh has slope
    1 so it keeps full accuracy; b_hn folded via an fp16 const group in
    hb). t1 = r*h_n_psum is one merged DVE multiply; the i_n + t1 add
    rides the PE as an fp16 identity-matmul accumulate; tanh drains it.
  - Blend (1-z)n + z*hs: d/e on DVE (fp16 2x mode), the terminal ho add
    on the otherwise-idle GPSIMD for early tiles.
  - Schedule: all input DMAs issued up front (lead tile + all rz weights
    in ONE fp8 DMA); ~3us of PE warm-up matmuls beat the p-state ramp;
    per tile the PE stream is [rz, hn, in(t-1), id(t-1)] and the Act
    stream [R, B(t-1), Z] so the tanh chain never starves the sigmoids;
    the last two tiles' tanh/blend/output run as 250-col halves into a
    dedicated contiguous output tensor so the tail drains in small
    overlapping chunks.
"""

import numpy as np

N, H, IN2, M, NCORES = 100000, 256, 128, 20000, 8
MC = M // NCORES        # 2500 observed rows per core
RT = 500                # rows per column-tile (<=512 fp32 PSUM bank limit)
NRT = MC // RT          # 5 row tiles per core
S = 16.0                # fp8 weight pre-scale; activations apply 1/S

_compiled = {}


def _build_nc():
    from contextlib import ExitStack

    from concourse import bacc
    import concourse.mybir as mybir
    from concourse.tile import TileContext

    dt = mybir.dt
    f32 = dt.float32
    f16 = dt.float16
    f8 = dt.float8e4
    AF = mybir.ActivationFunctionType
    ALU = mybir.AluOpType
    DR = mybir.MatmulPerfMode.DoubleRow

    nc = bacc.Bacc(None, target_bir_lowering=False)

    u_d = nc.dram_tensor("u", [NRT, 128, 4, RT], f8, kind="ExternalInput")
    hb_d = nc.dram_tensor("hb", [NRT, 128, 3, RT], f16, kind="ExternalInput")
    # uw0: tile-0 moving pairs [x|const, h_lo|h_hi] in cols [0:1000] plus ALL
    # rz weight pairs (A in [1024:1536], B in [1536:2048]; 16B-aligned
    # steps/offsets for the DoubleRow Ldweights ISA check) -> one lead DMA
    uw0_d = nc.dram_tensor("uw0", [128, 2, 2048], f8, kind="ExternalInput")
    win_d = nc.dram_tensor("win", [128, 4, 128], f8, kind="ExternalInput")
    whn_d = nc.dram_tensor("whn", [128, 6, 128], f16, kind="ExternalInput")
    id_d = nc.dram_tensor("ident", [128, 128], f16, kind="ExternalInput")
    out_d = nc.dram_tensor("hout", [NRT, 128, 2, RT], f16, kind="ExternalOutput")
    otl_d = nc.dram_tensor("houttl", [4, 128, 2, RT // 2], f16, kind="ExternalOutput")

    with TileContext(nc) as tc, ExitStack() as ctx:
        const = ctx.enter_context(tc.tile_pool(name="const", bufs=1))
        uin = ctx.enter_context(tc.tile_pool(name="uin", bufs=NRT))
        hin = ctx.enter_context(tc.tile_pool(name="hin", bufs=NRT))
        rzp = ctx.enter_context(tc.tile_pool(name="rzp", bufs=NRT))
        wrk = ctx.enter_context(tc.tile_pool(name="wrk", bufs=2))
        outp = ctx.enter_context(tc.tile_pool(name="outp", bufs=NRT))
        psum = ctx.enter_context(tc.tile_pool(name="psum", bufs=1, space="PSUM"))

        # --- all input DMAs up front (no WAR waits: bufs=NRT), critical first ---
        u_t = [None] * NRT
        hb_t = [None] * NRT
        uw0_sb = const.tile([128, 2, 2048], f8, tag="uw0")
        nc.sync.dma_start(out=uw0_sb[:], in_=uw0_d[:, :, :])
        hb_t[0] = hin.tile([128, 3, RT], f16, tag="hb", name="hb0")
        nc.gpsimd.dma_start(out=hb_t[0][:], in_=hb_d[0])
        whn_sb = const.tile([128, 6, 128], f16, tag="whn")
        nc.sync.dma_start(out=whn_sb[:], in_=whn_d[:, :, :])
        u_t[1] = uin.tile([128, 4, RT], f8, tag="u", name="u1")
        nc.sync.dma_start(out=u_t[1][:], in_=u_d[1])
        win_sb = const.tile([128, 4, 128], f8, tag="win")
        nc.sync.dma_start(out=win_sb[:], in_=win_d[:, :, :])
        id_sb = const.tile([128, 128], f16, tag="ident")
        nc.sync.dma_start(out=id_sb[:], in_=id_d[:, :])
        for tt in range(1, NRT):
            if tt > 1:
                u_t[tt] = uin.tile([128, 4, RT], f8, tag="u", name=f"u{tt}")
                nc.sync.dma_start(out=u_t[tt][:], in_=u_d[tt])
            hb_t[tt] = hin.tile([128, 3, RT], f16, tag="hb", name=f"hb{tt}")
            nc.sync.dma_start(out=hb_t[tt][:], in_=hb_d[tt])

        # --- PE warm-up: ~3us of throwaway matmuls so the p-state model is
        # at full clock when the first real matmuls arrive. Uses the hnps
        # tag (its first real use is latest) so the WAR chain clears early.
        wdum = const.tile([128, 256], f16, tag="wdum")
        nc.vector.memset(wdum[:], 0.0)
        warm_ps = psum.tile([128, 2, 512], f32, tag="hnps", name="warm")
        for i in range(12):
            nc.tensor.matmul(
                warm_ps[:, 0, 0:256], lhsT=wdum[:, 0:128], rhs=wdum[:],
                start=True, stop=True,
            )

        e_hold = [None]
        r_t = [None] * NRT
        z_t = [None] * NRT
        n_t = [None] * NRT
        t1_t = [[None, None] for _ in range(NRT)]
        inps_t = [None] * NRT
        ho_t = [None] * (NRT + 1)

        def u_pair(t, which):
            # moving pair AP [128, 2, RT]: which=0 -> (x, const), 1 -> (h0, h1)
            if t == 0:
                return uw0_sb[:, :, which * RT : (which + 1) * RT]
            return u_t[t][:, 2 * which : 2 * which + 2, :]

        def rz_mms(t, gh, ps):
            # gh in 0..3 -> r0,r1,z0,z1; two DoubleRow matmuls: (x,const)+(h0,h1)
            nc.tensor.matmul(
                ps[:], lhsT=uw0_sb[:, :, 1024 + gh * 128 : 1152 + gh * 128],
                rhs=u_pair(t, 0), start=True, stop=False, perf_mode=DR,
            )
            nc.tensor.matmul(
                ps[:], lhsT=uw0_sb[:, :, 1536 + gh * 128 : 1664 + gh * 128],
                rhs=u_pair(t, 1), start=False, stop=True, perf_mode=DR,
            )

        def id_mms(t):
            # i_n PSUM accumulation finishes with the t1 identity-matmul ride
            for j in range(2):
                nc.tensor.matmul(
                    inps_t[t][:, j, 0:RT], lhsT=id_sb[:], rhs=t1_t[t][j],
                    start=False, stop=True,
                )

        def emit_in_id(tp, tag="inps"):
            # i_n DoubleRow matmuls for tile tp, then the t1 identity ride.
            # Emitted one section late so the wait on B_{tp-1}'s PSUM free
            # never blocks the next tile's r/z matmuls in the PE stream.
            inps_t[tp] = psum.tile([128, 2, 512], f32, tag=tag, name="inps")
            for j in range(2):
                nc.tensor.matmul(
                    inps_t[tp][:, j, 0:RT], lhsT=win_sb[:, 2 * j : 2 * j + 2, :],
                    rhs=u_pair(tp, 0), start=True, stop=False, perf_mode=DR,
                )
            id_mms(tp)

        def emit_b(tp):
            n_t[tp] = wrk.tile([128, 2, RT], f16, tag="n", name="n_t", bufs=3)
            nc.scalar.activation(
                out=n_t[tp][:], in_=inps_t[tp][:, 0:2, 0:RT],
                func=AF.Tanh, scale=1.0 / S,
            )

        def emit_blend(tp):
            # d/e on DVE (they feed the DVE-ordered chain); ho is terminal so
            # early tiles push it to the idle GPSIMD engine. Late tiles keep
            # everything on DVE so the kernel tail is short.
            d_t = wrk.tile([128, 2, RT], f16, tag="d", name="d_t", bufs=3)
            nc.vector.tensor_sub(
                out=d_t[:], in0=hb_t[tp][:, 0:2, :], in1=n_t[tp][:]
            )
            e_t = wrk.tile([128, 2, RT], f16, tag="e", name="e_t", bufs=3)
            nc.vector.tensor_mul(out=e_t[:], in0=z_t[tp], in1=d_t[:])
            ho_t[tp] = outp.tile([128, 2, RT], f16, tag="ho", name="ho")
            if tp < NRT - 2:
                nc.gpsimd.tensor_add(out=ho_t[tp][:], in0=n_t[tp][:], in1=e_t[:])
            else:
                # deferred: emitted in the tail AFTER the last tile's chain so
                # the final output DMA is never stuck behind it
                e_hold[0] = (tp, e_t)

        for t in range(NRT):
            # --- PE: r/z first (feed Act asap), then prior tile's n-path ---
            rps = psum.tile([128, 2, 512], f32, tag="rps", name="rps")
            zps = psum.tile([128, 2, 512], f32, tag="zps", name="zps")
            for j in range(2):
                rz_mms(t, j, rps[:, j, 0:RT])
            for j in range(2):
                rz_mms(t, 2 + j, zps[:, j, 0:RT])
            hnps = psum.tile([128, 2, 512], f32, tag="hnps", name="hnps")
            for j in range(2):
                for k in range(3):
                    nc.tensor.matmul(
                        hnps[:, j, 0:RT], lhsT=whn_sb[:, 3 * j + k, :],
                        rhs=hb_t[t][:, k, :], start=(k == 0), stop=(k == 2),
                    )
            if t > 0:
                emit_in_id(t - 1)

            # --- Act: R_t, then B_{t-1}, then Z_t. Pulling the tanh one
            # slot earlier lets each tile's blend chain start ~1us sooner
            # (Z_t is only needed by the blend, which follows B anyway).
            r_sb = rzp.tile([128, 2, RT], f16, tag="r", name="r_sb")
            nc.scalar.activation(
                out=r_sb[:], in_=rps[:, :, 0:RT], func=AF.Sigmoid, scale=1.0 / S
            )
            r_t[t] = r_sb[:]
            if t > 0 and t - 1 != NRT - 2:
                emit_b(t - 1)
            z_sb = rzp.tile([128, 2, RT], f16, tag="z", name="z_sb")
            nc.scalar.activation(
                out=z_sb[:], in_=zps[:, :, 0:RT], func=AF.Sigmoid, scale=1.0 / S
            )
            z_t[t] = z_sb[:]

            # --- DVE: t1 = h_n_psum * r, one merged op (bias is in PSUM).
            # The last tile's t1 is computed in halves by the tail instead.
            if t < NRT - 1:
                t1m = wrk.tile([128, 2, RT], f16, tag="t1", name="t1", bufs=3)
                nc.vector.tensor_mul(
                    out=t1m[:], in0=hnps[:, :, 0:RT], in1=r_t[t]
                )
                t1_t[t] = [t1m[:, 0, :], t1m[:, 1, :]]

            # --- blend of previous tile: ho = n + z*(hb - n) ---
            if t > 0 and t - 1 != NRT - 2:
                emit_blend(t - 1)

        # --- tail: the last TWO tiles' tanh/blend/output run as 250-col
        # halves so each chunk's DMA overlaps the next chunk's compute and
        # the final transfers are small. Tile NRT-2's i_n/id ride stays
        # full-width (emitted in the last loop section); tile NRT-1's is
        # done per-half on the freed rps/zps banks. ---
        t3 = NRT - 2
        t = NRT - 1
        HC = RT // 2
        ho_h = [None] * 4
        for hh in range(2):
            cs = slice(hh * HC, (hh + 1) * HC)
            n_h = wrk.tile([128, 2, HC], f16, tag=f"n3h{hh}", name="n3h")
            nc.scalar.activation(
                out=n_h[:], in_=inps_t[t3][:, 0:2, cs], func=AF.Tanh,
                scale=1.0 / S,
            )
            dh = wrk.tile([128, 2, HC], f16, tag=f"d3h{hh}", name="d3h")
            nc.vector.tensor_sub(out=dh[:], in0=hb_t[t3][:, 0:2, cs], in1=n_h[:])
            eh = wrk.tile([128, 2, HC], f16, tag=f"e3h{hh}", name="e3h")
            nc.vector.tensor_mul(out=eh[:], in0=z_t[t3][:, :, cs], in1=dh[:])
            ho_h[hh] = outp.tile([128, 2, HC], f16, tag=f"ho3h{hh}", name="ho3h")
            nc.vector.tensor_add(out=ho_h[hh][:], in0=n_h[:], in1=eh[:])
        for hh in range(2):
            cs = slice(hh * HC, (hh + 1) * HC)
            ptag = "rps" if hh == 0 else "zps"
            inps_h = psum.tile([128, 2, 512], f32, tag=ptag, name=f"inps_t{hh}")
            t1h = wrk.tile([128, 2, HC], f16, tag=f"t1h{hh}", name="t1h")
            nc.vector.tensor_mul(
                out=t1h[:], in0=hnps[:, :, cs], in1=r_t[t][:, :, cs]
            )
            for j2 in range(2):
                nc.tensor.matmul(
                    inps_h[:, j2, 0:HC], lhsT=win_sb[:, 2 * j2 : 2 * j2 + 2, :],
                    rhs=u_pair(t, 0)[:, :, cs], start=True, stop=False,
                    perf_mode=DR,
                )
                nc.tensor.matmul(
                    inps_h[:, j2, 0:HC], lhsT=id_sb[:], rhs=t1h[:, j2, :],
                    start=False, stop=True,
                )
            n_h = wrk.tile([128, 2, HC], f16, tag=f"ntail{hh}", name="n_h")
            nc.scalar.activation(
                out=n_h[:], in_=inps_h[:, 0:2, 0:HC], func=AF.Tanh, scale=1.0 / S
            )
            dh = wrk.tile([128, 2, HC], f16, tag=f"dh{hh}", name="dh")
            nc.vector.tensor_sub(out=dh[:], in0=hb_t[t][:, 0:2, cs], in1=n_h[:])
            eh = wrk.tile([128, 2, HC], f16, tag=f"eh{hh}", name="eh")
            nc.vector.tensor_mul(out=eh[:], in0=z_t[t][:, :, cs], in1=dh[:])
            ho_h[2 + hh] = outp.tile([128, 2, HC], f16, tag=f"hoh{hh}", name="ho_h")
            nc.vector.tensor_add(out=ho_h[2 + hh][:], in0=n_h[:], in1=eh[:])
        # out DMAs last on SP so their sem-waits never block input DMA issue
        for tt in range(NRT - 2):
            nc.gpsimd.dma_start(out=out_d[tt], in_=ho_t[tt][:])
        for q in range(4):
            nc.sync.dma_start(out=otl_d[q], in_=ho_h[q][:])

    nc.compile()
    return nc


def _get_nc():
    if "nc" not in _compiled:
        _compiled["nc"] = _build_nc()
    return _compiled["nc"]


def _make_in_maps(h, X_obs, i_obs, W_ih, W_hh, b_ih, b_hh):
    import ml_dtypes

    f32 = np.float32
    f16 = np.float16
    f8 = ml_dtypes.float8_e4m3

    x = np.asarray(X_obs, f32).reshape(M, IN2)
    hs = np.asarray(h, f32)[np.asarray(i_obs)]
    W_ih = np.asarray(W_ih, f32)
    W_hh = np.asarray(W_hh, f32)
    b_ih = np.asarray(b_ih, f32)
    b_hh = np.asarray(b_hh, f32)

    wiT = W_ih.T * S          # [128, 768]
    whT = W_hh.T * S          # [256, 768]
    brz = (b_ih[: 2 * H] + b_hh[: 2 * H]) * S    # [512]
    bin_ = b_ih[2 * H :] * S                     # [256]
    bhn = b_hh[2 * H :] * S                      # [256]

    # rz weights as DoubleRow pairs [128, 2, 128] per gate-half:
    # A-pair (Wih-block, bias-block), B-pair (Whh-lo, Whh-hi); packed into
    # the uw0 lead tile at cols [1000:1512] (A) and [1512:2024] (B).
    wrzA = np.zeros((128, 2, 512), f32)
    wrzB = np.zeros((128, 2, 512), f32)
    for gh in range(4):
        gs = slice(gh * 128, (gh + 1) * 128)
        cs = slice(gh * 128, (gh + 1) * 128)
        wrzA[:, 0, cs] = wiT[:, gs]
        wrzA[0, 1, cs] = brz[gs]
        wrzB[:, 0, cs] = whT[0:128, gs]
        wrzB[:, 1, cs] = whT[128:256, gs]
    # win: per half j: [Wih_n-block, bias-block]
    win = np.zeros((128, 4, 128), f32)
    for j in range(2):
        gs = slice(2 * H + j * 128, 2 * H + (j + 1) * 128)
        win[:, 2 * j + 0, :] = wiT[:, gs]
        win[0, 2 * j + 1, :] = bin_[j * 128 : (j + 1) * 128]
    # whn: per half j: [Whh_n lo-block, hi-block, bias-block]  (fp16)
    whn = np.zeros((128, 6, 128), f32)
    for j in range(2):
        gs = slice(2 * H + j * 128, 2 * H + (j + 1) * 128)
        whn[:, 3 * j + 0, :] = whT[0:128, gs]
        whn[:, 3 * j + 1, :] = whT[128:256, gs]
        whn[0, 3 * j + 2, :] = bhn[j * 128 : (j + 1) * 128]

    win = win.astype(f8)
    whn = whn.astype(f16)
    ident = np.eye(128, dtype=f16)

    xT = x.T                   # [128, M]
    hT = hs.T                  # [256, M]
    in_maps = []
    for c in range(NCORES):
        cols = slice(c * MC, (c + 1) * MC)
        xc = xT[:, cols]       # [128, MC]
        hc = hT[:, cols]       # [256, MC]
        # u: [NRT, 128, 4, RT] groups (x, const, h_lo, h_hi) in fp8
        u = np.zeros((NRT, 128, 4, RT), f32)
        hb = np.zeros((NRT, 128, 3, RT), f32)
        for t in range(NRT):
            cs = slice(t * RT, (t + 1) * RT)
            u[t, :, 0, :] = xc[:, cs]
            u[t, 0, 1, :] = 1.0
            u[t, :, 2, :] = hc[0:128, cs]
            u[t, :, 3, :] = hc[128:256, cs]
            hb[t, :, 0, :] = hc[0:128, cs]
            hb[t, :, 1, :] = hc[128:256, cs]
            hb[t, 0, 2, :] = 1.0
        # uw0: tile-0 moving pairs + all rz weight pairs in one lead DMA
        uw0 = np.zeros((128, 2, 2048), f32)
        uw0[:, 0, 0:RT] = u[0, :, 0, :]
        uw0[:, 1, 0:RT] = u[0, :, 1, :]
        uw0[:, 0, RT : 2 * RT] = u[0, :, 2, :]
        uw0[:, 1, RT : 2 * RT] = u[0, :, 3, :]
        uw0[:, :, 1024:1536] = wrzA
        uw0[:, :, 1536:2048] = wrzB
        in_maps.append(
            {
                "u": u.astype(f8),
                "hb": hb.astype(f16),
                "uw0": uw0.astype(f8),
                "win": win,
                "whn": whn,
                "ident": ident,
            }
        )
    return in_maps


def run_on_device(h, X_obs, i_obs, W_ih, W_hh, b_ih, b_hh, **run_kwargs):
    """Returns (h_new [M,H] fp32, BassKernelResults)."""
    from concourse.bass_utils import run_bass_kernel_spmd

    in_maps = _make_in_maps(h, X_obs, i_obs, W_ih, W_hh, b_ih, b_hh)
    res = run_bass_kernel_spmd(_get_nc(), in_maps, list(range(NCORES)), **run_kwargs)
    parts = []
    for r in res.results:
        o = np.asarray(r["hout"], np.float32)   # [NRT, 128, 2, RT]
        tl = np.asarray(r["houttl"], np.float32)  # [4, 128, 2, RT//2]
        o[NRT - 2, :, :, 0 : RT // 2] = tl[0]
        o[NRT - 2, :, :, RT // 2 : RT] = tl[1]
        o[NRT - 1, :, :, 0 : RT // 2] = tl[2]
        o[NRT - 1, :, :, RT // 2 : RT] = tl[3]
        # [t, p, j, c] -> rows t*RT+c, dims j*128+p
        o = o.transpose(0, 3, 2, 1).reshape(MC, H)
        parts.append(o)
    h_new = np.concatenate(parts, axis=0)
    return h_new, res


def kernel(h, X_obs, i_obs, W_ih, W_hh, b_ih, b_hh):
    h = np.asarray(h, np.float32)
    i_obs = np.asarray(i_obs)
    h_new, _ = run_on_device(h, X_obs, i_obs, W_ih, W_hh, b_ih, b_hh)
    out = h.copy()
    out[i_obs] = h_new
    return out
